# revision 1
# baseline (speedup 1.0000x reference)
"""SAGAN-style self-attention on 8 trn2 cores: data-parallel over batch.

Per core (one batch image): x^T [256,4096] bf16 in, out^T [256,4096] f32 out.
  QT/KT = W^T @ xT          [32, 4096]   (bias via K=1 ones-matmul preload)
  V     = x @ Wh + bh       [4096, 256]  ([keys, c] layout, 32 tiles of 128)
  per 512-query tile, per 128-key block:
    T    = KT_blk.T @ QT_tile   [128 keys, 512 queries]  (K=32 contraction)
    expT = exp(T)               ScalarE, PSUM->SBUF, bf16, no max-subtraction
                                (scores ~N(0, 0.58^2), |s| < ~5 -> fp32-safe)
    O'  += V_blk.T @ expT       [256, 512] PSUM accum over 32 key blocks
    Z   += ones.T @ expT        [1, 512]   softmax denominator
  zr = 1/Z -> broadcast to 128 partitions via K=1 matmul
  O = O' * zr;  out^T = Wo.T @ O + bo;  DMA out.
"""

import sys

if "/opt/trn_rl_repo" not in sys.path:
    sys.path.insert(0, "/opt/trn_rl_repo")

import ml_dtypes
import numpy as np

import concourse.bass as bass
import concourse.mybir as mybir
import concourse.tile as tile
from concourse.bass_utils import run_bass_kernel_spmd

B, H, W, C = 8, 64, 64, 256
KEY = 32
N = H * W          # 4096 tokens
NT = 512           # query tile (free dim per matmul)
NTILES = N // NT   # 8
MB = 128           # key block (contraction chunk)
NMB = N // MB      # 32
GRP = 4            # key blocks per group: one per PE row strip

BF16 = mybir.dt.bfloat16
F32 = mybir.dt.float32
FT = mybir.ActivationFunctionType


def build_nc() -> bass.Bass:
    nc = bass.Bass()

    xT = nc.declare_dram_parameter("xT", [2, 128, N], BF16, isOutput=False)
    wf = nc.declare_dram_parameter("wf", [2, 128, KEY], BF16, isOutput=False)
    wg = nc.declare_dram_parameter("wg", [2, 128, KEY], BF16, isOutput=False)
    wh = nc.declare_dram_parameter("wh", [2, 128, C], BF16, isOutput=False)
    wo = nc.declare_dram_parameter("wo", [2, 128, C], BF16, isOutput=False)
    bfp = nc.declare_dram_parameter("bfp", [1, KEY], BF16, isOutput=False)
    bgp = nc.declare_dram_parameter("bgp", [1, KEY], BF16, isOutput=False)
    bhp = nc.declare_dram_parameter("bhp", [1, C], BF16, isOutput=False)
    bop = nc.declare_dram_parameter("bop", [1, C], BF16, isOutput=False)
    outT = nc.declare_dram_parameter("outT", [2, 128, N], F32, isOutput=True)

    with tile.TileContext(nc) as tc:
        with (
            tc.tile_pool(name="const", bufs=1) as const,
            tc.tile_pool(name="xp", bufs=1) as xp,
            tc.tile_pool(name="vp", bufs=1) as vp,
            tc.tile_pool(name="qk", bufs=1) as qk,
            tc.tile_pool(name="ep", bufs=3) as ep,
            tc.tile_pool(name="osb", bufs=2) as osbp,
            tc.tile_pool(name="zp", bufs=2) as zp,
            tc.tile_pool(name="outp", bufs=3) as outp,
            # PSUM budget is 8 banks total (statically reserved per pool/tag):
            #   pt: tag "t"  [128,2048]f32 x1      = 4 banks (exp of group g
            #       overlaps the 12 O'/Z matmuls of group g, so single-buffer
            #       is stall-free)
            #   po: tags o0/o1 [128,512]f32 x1each = 2 banks (outproj shares)
            #   pz: tags z + zb                    = 2 banks
            # Projections borrow "t"/"zb" slots before attention starts.
            tc.tile_pool(name="pt", bufs=1, space="PSUM") as pt,
            tc.tile_pool(name="po", bufs=1, space="PSUM") as po,
            tc.tile_pool(name="pz", bufs=1, space="PSUM") as pz,
        ):
            # ---- constants ----
            ones_col = const.tile([128, 1], BF16)   # Z-matmul lhsT
            ones_m = const.tile([1, 128], BF16)     # K=1 broadcast lhsT
            ones_m32 = const.tile([1, 128], F32)    # K=1 broadcast lhsT (f32)
            ones_r = const.tile([1, NT], BF16)      # K=1 bias-preload rhs
            nc.vector.memset(ones_col, 1.0)
            nc.vector.memset(ones_m, 1.0)
            nc.vector.memset(ones_m32, 1.0)
            nc.vector.memset(ones_r, 1.0)

            wf_sb = const.tile([128, 2, KEY], BF16)
            wg_sb = const.tile([128, 2, KEY], BF16)
            wh_sb = const.tile([128, 2, C], BF16)
            wo_sb = const.tile([128, 2, C], BF16)
            bf_sb = const.tile([1, KEY], BF16)
            bg_sb = const.tile([1, KEY], BF16)
            bh_sb = const.tile([1, C], BF16)
            bo_sb = const.tile([1, C], BF16)
            for cc in range(2):
                nc.sync.dma_start(out=wf_sb[:, cc, :], in_=wf[cc])
                nc.sync.dma_start(out=wg_sb[:, cc, :], in_=wg[cc])
                nc.sync.dma_start(out=wh_sb[:, cc, :], in_=wh[cc])
                nc.sync.dma_start(out=wo_sb[:, cc, :], in_=wo[cc])
            nc.sync.dma_start(out=bf_sb, in_=bfp[:])
            nc.sync.dma_start(out=bg_sb, in_=bgp[:])
            nc.sync.dma_start(out=bh_sb, in_=bhp[:])
            nc.sync.dma_start(out=bo_sb, in_=bop[:])

            # xT chunks, split into 512-column tiles so projections start as
            # soon as the first slice lands (and spread across DMA queues)
            xts = [
                [xp.tile([128, NT], BF16, name=f"xt{cc}_{h}") for h in range(NTILES)]
                for cc in range(2)
            ]
            for h in range(NTILES):
                for cc in range(2):
                    nc.sync.dma_start(
                        out=xts[cc][h], in_=xT[cc, :, h * NT:(h + 1) * NT]
                    )

            def xs(cc, start, width):
                # column slice of xT chunk cc; never crosses a tile boundary
                h = start // NT
                assert (start + width - 1) // NT == h
                return xts[cc][h][:, start - h * NT: start - h * NT + width]

            pp_i = 0

            def proj_psum(shape):
                # alternate between the two borrowed slots for double-buffering
                nonlocal pp_i
                pp_i += 1
                if pp_i % 2:
                    return pt.tile(shape, F32, tag="t", name=f"projps{pp_i}")
                return pz.tile(shape, F32, tag="zb", name=f"projps{pp_i}")

            # ---- Q/K projections ----
            # qt_rep [128, N]: Q^T replicated at the four 32-row strip offsets
            # (each row-packed T matmul streams its rhs from its strip's
            # partitions). Strip 0 written by ACT from PSUM, strips 1-3 by
            # SBUF->SBUF DMA.
            qt_rep = qk.tile([128, N], BF16)
            kt = qk.tile([KEY, N], BF16)
            for g in range(NTILES):
                sl = slice(g * NT, (g + 1) * NT)
                for dst, w_sb, b_sb in ((qt_rep, wf_sb, bf_sb), (kt, wg_sb, bg_sb)):
                    ps = proj_psum([KEY, NT])
                    nc.tensor.matmul(ps, b_sb, ones_r, start=True, stop=False)
                    for cc in range(2):
                        nc.tensor.matmul(
                            ps, w_sb[:, cc, :], xs(cc, g * NT, NT),
                            start=False, stop=(cc == 1),
                        )
                    nc.scalar.copy(out=dst[0:KEY, sl], in_=ps)
            for i in range(1, 4):
                nc.sync.dma_start(
                    out=qt_rep[32 * i:32 * (i + 1), :], in_=qt_rep[0:KEY, :]
                )
            # kt_stack [128, NMB//4, 128]: strip i of group g holds
            # K^T[:, (4g+i)*128:(4g+i+1)*128] — stationary operands for the
            # 4-way row-packed T matmuls. Built by small regrouping DMAs.
            kt_stack = qk.tile([128, NMB // 4, MB], BF16)
            for g in range(NMB // 4):
                for i in range(4):
                    b = 4 * g + i
                    nc.sync.dma_start(
                        out=kt_stack[32 * i:32 * (i + 1), g, :],
                        in_=kt[:, b * MB:(b + 1) * MB],
                    )

            # ---- V projection -> 32 tiles [128, 256] bf16 ([keys, c]) ----
            v_sb = []
            for mb in range(NMB):
                ps = proj_psum([128, C])
                nc.tensor.matmul(ps, ones_m, bh_sb, start=True, stop=False)
                for cc in range(2):
                    nc.tensor.matmul(
                        ps, xs(cc, mb * MB, MB), wh_sb[:, cc, :],
                        start=False, stop=(cc == 1),
                    )
                vt = vp.tile([128, C], BF16, tag=f"v{mb}")
                nc.vector.tensor_copy(out=vt, in_=ps)
                v_sb.append(vt)

            # ---- attention: flat software pipeline over (query-tile, group) ----
            NGRP = NMB // GRP  # 8 groups of 4 key blocks per query tile
            cur = {}           # nt -> (o_ps pair, z_ps)

            def emit_tail(nt):
                """normalize + output projection + store for query tile nt"""
                o_ps, z_ps = cur.pop(nt)
                nsl = slice(nt * NT, (nt + 1) * NT)
                zr_sb = zp.tile([1, NT], F32, tag="zr", name=f"zr{nt}")
                nc.vector.reciprocal(out=zr_sb, in_=z_ps)
                zb_ps = pz.tile([128, NT], F32, tag="zb", name=f"zb{nt}")
                nc.tensor.matmul(zb_ps, ones_m32, zr_sb, start=True, stop=True)
                zb_sb = zp.tile([128, NT], F32, tag="zb_sb", name=f"zbs{nt}")
                nc.vector.tensor_copy(out=zb_sb, in_=zb_ps)
                osb = []
                for cc in range(2):
                    ot = osbp.tile([128, NT], BF16, tag=f"os{cc}", name=f"os{cc}_{nt}")
                    nc.vector.tensor_mul(ot, o_ps[cc], zb_sb)
                    osb.append(ot)
                # out^T[c',n] = sum_c Wo[c,c'] O[c,n] + bo[c']
                for cp in range(2):
                    csl = slice(cp * 128, (cp + 1) * 128)
                    f_ps = po.tile([128, NT], F32, tag=f"o{cp}", name=f"f{cp}_{nt}")
                    nc.tensor.matmul(
                        f_ps, bo_sb[:, csl], ones_r, start=True, stop=False,
                    )
                    for cc in range(2):
                        nc.tensor.matmul(
                            f_ps, wo_sb[:, cc, csl], osb[cc],
                            start=False, stop=(cc == 1),
                        )
                    out_sb = outp.tile([128, NT], F32, tag="out", name=f"out{cp}_{nt}")
                    nc.scalar.copy(out=out_sb, in_=f_ps)
                    nc.sync.dma_start(out=outT[cp, :, nsl], in_=out_sb)

            def emit_oz(nt, g, e_sb):
                """O'/Z accumulation for group g of tile nt (+tail after last)"""
                if g == 0:
                    cur[nt] = (
                        [po.tile([128, NT], F32, tag="o0", name=f"o0_{nt}"),
                         po.tile([128, NT], F32, tag="o1", name=f"o1_{nt}")],
                        pz.tile([1, NT], F32, tag="z", name=f"z{nt}"),
                    )
                o_ps, z_ps = cur[nt]
                for j in range(GRP):
                    mb = g * GRP + j
                    esl = e_sb[:, j * NT:(j + 1) * NT]
                    first, last = mb == 0, mb == NMB - 1
                    for cc in range(2):
                        nc.tensor.matmul(
                            o_ps[cc],
                            v_sb[mb][:, cc * 128:(cc + 1) * 128],
                            esl,
                            start=first, stop=last,
                        )
                    nc.tensor.matmul(
                        z_ps, ones_col, esl, start=first, stop=last,
                    )
                if g == NGRP - 1:
                    emit_tail(nt)

            # Pipelined one group deep: T-pack(i) ... O'/Z(i-1). exp(i) on ACT
            # hides under O'/Z(i-1) on PE; the per-tile tail (DVE-heavy) hides
            # under the next tile's first T-pack/O'Z groups.
            prev = None
            for nt in range(NTILES):
                nsl = slice(nt * NT, (nt + 1) * NT)
                for g in range(NGRP):
                    # 4-way row-packed score matmuls: strip j contracts its own
                    # 32 rows of the PE array concurrently (measured ~3x).
                    t_ps = pt.tile([128, GRP * NT], F32, tag="t", name=f"t{nt}_{g}")
                    for j in range(GRP):
                        nc.tensor.matmul(
                            t_ps[:, j * NT:(j + 1) * NT],
                            kt_stack[32 * j:32 * (j + 1), g, :],
                            qt_rep[32 * j:32 * (j + 1), nsl],
                            start=True, stop=True,
                            tile_position=(32 * j, 0),
                        )
                    e_sb = ep.tile([128, GRP * NT], BF16, tag="e", name=f"e{nt}_{g}")
                    nc.scalar.activation(out=e_sb, in_=t_ps, func=FT.Exp)
                    if prev is not None:
                        emit_oz(*prev)
                    prev = (nt, g, e_sb)
            emit_oz(*prev)

    _split_multiwaits(nc)
    return nc


def _split_multiwaits(nc: bass.Bass) -> None:
    """This container's walrus accepts at most ONE sync-wait per instruction
    (CoreV3GenImpl setupSyncWait). Tile emits multi-wait instructions; split
    the excess waits onto EventSemaphore carriers inserted just before the
    instruction on the same engine — same-engine program order makes this
    semantics-preserving."""
    import json as _json

    data = _json.loads(mybir.module_to_json_bytes(nc.m))
    uid = 0
    for fn in data["functions"]:
        for bb in fn["blocks"]:
            new = []
            for inst in bb["instructions"]:
                si = inst.get("sync_info")
                waits = (si or {}).get("on_wait") or []
                if len(waits) > 1:
                    for wcmd in waits[:-1]:
                        uid += 1
                        new.append({
                            "debug": inst.get("debug", 0),
                            "engine": inst["engine"],
                            "ins": [], "outs": [],
                            "name": f"syncw-{uid}",
                            "opcode": "EventSemaphore",
                            "sync_info": {"on_update": [], "on_wait": [wcmd]},
                        })
                    si["on_wait"] = [waits[-1]]
                new.append(inst)
            bb["instructions"] = new
    nc.m = mybir.module_from_json_bytes(_json.dumps(data).encode())


_NC = None


def _get_nc():
    global _NC
    if _NC is None:
        _NC = build_nc()
    return _NC


def _prep_maps(x, Wf, bf, Wg, bg, Wh, bh, Wo, bo):
    bft = ml_dtypes.bfloat16
    shared = {
        "wf": np.ascontiguousarray(Wf.reshape(2, 128, KEY).astype(bft)),
        "wg": np.ascontiguousarray(Wg.reshape(2, 128, KEY).astype(bft)),
        "wh": np.ascontiguousarray(Wh.reshape(2, 128, C).astype(bft)),
        "wo": np.ascontiguousarray(Wo.reshape(2, 128, C).astype(bft)),
        "bfp": np.ascontiguousarray(bf.reshape(1, KEY).astype(bft)),
        "bgp": np.ascontiguousarray(bg.reshape(1, KEY).astype(bft)),
        "bhp": np.ascontiguousarray(bh.reshape(1, C).astype(bft)),
        "bop": np.ascontiguousarray(bo.reshape(1, C).astype(bft)),
    }
    in_maps = []
    for b in range(B):
        xTb = np.ascontiguousarray(
            x[b].reshape(N, C).T.astype(bft).reshape(2, 128, N)
        )
        m = dict(shared)
        m["xT"] = xTb
        in_maps.append(m)
    return in_maps


def run(x, Wf, bf, Wg, bg, Wh, bh, Wo, bo, trace=False, **kw):
    x = np.asarray(x, dtype=np.float32)
    in_maps = _prep_maps(
        x, *(np.asarray(a, dtype=np.float32) for a in (Wf, bf, Wg, bg, Wh, bh, Wo, bo))
    )
    res = run_bass_kernel_spmd(_get_nc(), in_maps, list(range(B)), trace=trace, **kw)
    out = np.empty((B, H, W, C), dtype=np.float32)
    for b in range(B):
        oT = np.asarray(res.results[b]["outT"], dtype=np.float32).reshape(C, N)
        out[b] = oT.T.reshape(H, W, C)
    return out, res


def kernel(x, Wf, bf, Wg, bg, Wh, bh, Wo, bo):
    out, _ = run(x, Wf, bf, Wg, bg, Wh, bh, Wo, bo)
    return out



# revision 10
# speedup vs baseline: 1.1113x; 1.1113x over previous
"""SAGAN-style self-attention on 8 trn2 cores: data-parallel over batch.

Per core (one batch image): x^T [256,4096] bf16 in, out^T [256,4096] f32 out.
  QT/KT = W^T @ xT + b      [32, 4096]   (bias via ACT Identity per-partition)
  V     = x @ Wh            [4096, 256]  ([keys, c] layout; bh folds into bo')
  per 512-query tile, per 128-key-block group of 4:
    T    = KT_blk.T @ QT_tile   [128 keys, 4x512 queries]  (K=32, row-packed)
    expT = exp(T)               ScalarE, PSUM->SBUF, bf16, no max-subtraction
                                (scores ~N(0, 0.58^2), |s| < ~5 -> fp32-safe)
    O'  += V_blk.T @ expT       [256, 512] PSUM accum over 32 key blocks
    es   = e0+e1+e2+e3 (DVE);  Z += ones.T @ es  [1,512] one matmul per group
  zr = 1/Z -> broadcast to 128 partitions via K=1 matmul
  O = O' * zr;  out^T = Wo.T @ O + bo' (bo' = Wo.T bh + bo, ACT bias);  DMA.
"""

import sys

if "/opt/trn_rl_repo" not in sys.path:
    sys.path.insert(0, "/opt/trn_rl_repo")

import ml_dtypes
import numpy as np

import concourse.bass as bass
import concourse.mybir as mybir
import concourse.tile as tile
from concourse.bass_utils import run_bass_kernel_spmd

B, H, W, C = 8, 64, 64, 256
KEY = 32
N = H * W          # 4096 tokens
NT = 512           # query tile (free dim per matmul)
NTILES = N // NT   # 8
MB = 128           # key block (contraction chunk)
NMB = N // MB      # 32
GRP = 4            # key blocks per group: one per PE row strip

BF16 = mybir.dt.bfloat16
F32 = mybir.dt.float32
FT = mybir.ActivationFunctionType


def build_nc() -> bass.Bass:
    nc = bass.Bass()

    xT = nc.declare_dram_parameter("xT", [2, 128, N], BF16, isOutput=False)
    wf = nc.declare_dram_parameter("wf", [2, 128, KEY], BF16, isOutput=False)
    wg = nc.declare_dram_parameter("wg", [2, 128, KEY], BF16, isOutput=False)
    wh = nc.declare_dram_parameter("wh", [2, 128, C], BF16, isOutput=False)
    wo = nc.declare_dram_parameter("wo", [2, 128, C], BF16, isOutput=False)
    bfp = nc.declare_dram_parameter("bfp", [KEY, 1], F32, isOutput=False)
    bgp = nc.declare_dram_parameter("bgp", [KEY, 1], F32, isOutput=False)
    bop = nc.declare_dram_parameter("bop", [2, 128, 1], F32, isOutput=False)
    outT = nc.declare_dram_parameter("outT", [2, 128, N], F32, isOutput=True)

    with tile.TileContext(nc) as tc:
        with (
            tc.tile_pool(name="const", bufs=1) as const,
            tc.tile_pool(name="xp", bufs=1) as xp,
            tc.tile_pool(name="vp", bufs=1) as vp,
            tc.tile_pool(name="qk", bufs=1) as qk,
            tc.tile_pool(name="ep", bufs=3) as ep,
            tc.tile_pool(name="esp", bufs=2) as esp,
            tc.tile_pool(name="osb", bufs=2) as osbp,
            tc.tile_pool(name="zp", bufs=2) as zp,
            tc.tile_pool(name="outp", bufs=3) as outp,
            # PSUM budget is 8 banks total (statically reserved per pool/tag):
            #   pt: tag "t"  [128,2048]f32 x1      = 4 banks (exp of group g
            #       overlaps the 12 O'/Z matmuls of group g, so single-buffer
            #       is stall-free)
            #   po: tags o0/o1 [128,512]f32 x1each = 2 banks (outproj shares)
            #   pz: tags z + zb                    = 2 banks
            # Projections borrow "t"/"zb" slots before attention starts.
            tc.tile_pool(name="pt", bufs=1, space="PSUM") as pt,
            tc.tile_pool(name="po", bufs=1, space="PSUM") as po,
            tc.tile_pool(name="pz", bufs=1, space="PSUM") as pz,
        ):
            # ---- constants ----
            ones_col = const.tile([128, 1], BF16)   # Z-matmul lhsT
            ones_m32 = const.tile([1, 128], F32)    # K=1 broadcast lhsT (f32)
            nc.vector.memset(ones_col, 1.0)
            nc.vector.memset(ones_m32, 1.0)

            wf_sb = const.tile([128, 2, KEY], BF16)
            wg_sb = const.tile([128, 2, KEY], BF16)
            wh_sb = const.tile([128, 2, C], BF16)
            wo_sb = const.tile([128, 2, C], BF16)
            bf_sb = const.tile([KEY, 1], F32)
            bg_sb = const.tile([KEY, 1], F32)
            bo_sb = const.tile([128, 2], F32)
            for cc in range(2):
                nc.sync.dma_start(out=wf_sb[:, cc, :], in_=wf[cc])
                nc.sync.dma_start(out=wg_sb[:, cc, :], in_=wg[cc])
                nc.sync.dma_start(out=wh_sb[:, cc, :], in_=wh[cc])
                nc.sync.dma_start(out=wo_sb[:, cc, :], in_=wo[cc])
                nc.sync.dma_start(out=bo_sb[:, cc:cc + 1], in_=bop[cc])
            nc.sync.dma_start(out=bf_sb, in_=bfp[:])
            nc.sync.dma_start(out=bg_sb, in_=bgp[:])

            # xT chunks, split into 512-column tiles so projections start as
            # soon as the first slice lands (and spread across DMA queues)
            xts = [
                [xp.tile([128, NT], BF16, name=f"xt{cc}_{h}") for h in range(NTILES)]
                for cc in range(2)
            ]
            for h in range(NTILES):
                for cc in range(2):
                    nc.sync.dma_start(
                        out=xts[cc][h], in_=xT[cc, :, h * NT:(h + 1) * NT]
                    )

            def xs(cc, start, width):
                # column slice of xT chunk cc; never crosses a tile boundary
                h = start // NT
                assert (start + width - 1) // NT == h
                return xts[cc][h][:, start - h * NT: start - h * NT + width]

            pp_i = 0

            def proj_psum(shape):
                # alternate between the two borrowed slots for double-buffering
                nonlocal pp_i
                pp_i += 1
                if pp_i % 2:
                    return pt.tile(shape, F32, tag="t", name=f"projps{pp_i}")
                return pz.tile(shape, F32, tag="zb", name=f"projps{pp_i}")

            # ---- Q/K projections ----
            # qt_rep [128, N]: Q^T replicated at the four 32-row strip offsets
            # (each row-packed T matmul streams its rhs from its strip's
            # partitions). Strip 0 written by ACT from PSUM, strips 1-3 by
            # SBUF->SBUF DMA.
            qt_rep = qk.tile([128, N], BF16)
            kt = qk.tile([KEY, N], BF16)
            for g in range(NTILES):
                sl = slice(g * NT, (g + 1) * NT)
                for dst, w_sb, b_sb in ((qt_rep, wf_sb, bf_sb), (kt, wg_sb, bg_sb)):
                    ps = proj_psum([KEY, NT])
                    for cc in range(2):
                        nc.tensor.matmul(
                            ps, w_sb[:, cc, :], xs(cc, g * NT, NT),
                            start=(cc == 0), stop=(cc == 1),
                        )
                    nc.scalar.activation(
                        out=dst[0:KEY, sl], in_=ps, func=FT.Identity, bias=b_sb,
                    )
            for i in range(1, 4):
                nc.sync.dma_start(
                    out=qt_rep[32 * i:32 * (i + 1), :], in_=qt_rep[0:KEY, :]
                )
            # kt_stack [128, NMB//4, 128]: strip i of group g holds
            # K^T[:, (4g+i)*128:(4g+i+1)*128] — stationary operands for the
            # 4-way row-packed T matmuls. Built by small regrouping DMAs.
            kt_stack = qk.tile([128, NMB // 4, MB], BF16)
            for g in range(NMB // 4):
                for i in range(4):
                    b = 4 * g + i
                    nc.sync.dma_start(
                        out=kt_stack[32 * i:32 * (i + 1), g, :],
                        in_=kt[:, b * MB:(b + 1) * MB],
                    )

            # ---- V projection -> 32 tiles [128, 256] bf16 ([keys, c]) ----
            v_sb = []
            for mb in range(NMB):
                ps = proj_psum([128, C])
                for cc in range(2):
                    nc.tensor.matmul(
                        ps, xs(cc, mb * MB, MB), wh_sb[:, cc, :],
                        start=(cc == 0), stop=(cc == 1),
                    )
                vt = vp.tile([128, C], BF16, tag=f"v{mb}")
                nc.vector.tensor_copy(out=vt, in_=ps)
                v_sb.append(vt)

            # ---- attention: flat software pipeline over (query-tile, group) ----
            NGRP = NMB // GRP  # 8 groups of 4 key blocks per query tile
            cur = {}           # nt -> (o_ps pair, z_ps)

            def emit_tail(nt):
                """normalize + output projection + store for query tile nt"""
                o_ps, z_ps = cur.pop(nt)
                nsl = slice(nt * NT, (nt + 1) * NT)
                zr_sb = zp.tile([1, NT], F32, tag="zr", name=f"zr{nt}")
                nc.vector.reciprocal(out=zr_sb, in_=z_ps)
                zb_ps = pz.tile([128, NT], F32, tag="zb", name=f"zb{nt}")
                nc.tensor.matmul(zb_ps, ones_m32, zr_sb, start=True, stop=True)
                zb_sb = zp.tile([128, NT], F32, tag="zb_sb", name=f"zbs{nt}")
                nc.vector.tensor_copy(out=zb_sb, in_=zb_ps)
                osb = []
                for cc in range(2):
                    ot = osbp.tile([128, NT], BF16, tag=f"os{cc}", name=f"os{cc}_{nt}")
                    nc.vector.tensor_mul(ot, o_ps[cc], zb_sb)
                    osb.append(ot)
                # out^T[c',n] = sum_c Wo[c,c'] O[c,n] + bo'[c']
                for cp in range(2):
                    csl = slice(cp * 128, (cp + 1) * 128)
                    f_ps = po.tile([128, NT], F32, tag=f"o{cp}", name=f"f{cp}_{nt}")
                    for cc in range(2):
                        nc.tensor.matmul(
                            f_ps, wo_sb[:, cc, csl], osb[cc],
                            start=(cc == 0), stop=(cc == 1),
                        )
                    out_sb = outp.tile([128, NT], F32, tag="out", name=f"out{cp}_{nt}")
                    nc.scalar.activation(
                        out=out_sb, in_=f_ps, func=FT.Identity,
                        bias=bo_sb[:, cp:cp + 1],
                    )
                    nc.sync.dma_start(out=outT[cp, :, nsl], in_=out_sb)

            def emit_oz(nt, g, e_sb):
                """O'/Z accumulation for group g of tile nt (+tail after last)"""
                if g == 0:
                    cur[nt] = (
                        [po.tile([128, NT], F32, tag="o0", name=f"o0_{nt}"),
                         po.tile([128, NT], F32, tag="o1", name=f"o1_{nt}")],
                        pz.tile([1, NT], F32, tag="z", name=f"z{nt}"),
                    )
                o_ps, z_ps = cur[nt]
                for j in range(GRP):
                    mb = g * GRP + j
                    esl = e_sb[:, j * NT:(j + 1) * NT]
                    first, last = mb == 0, mb == NMB - 1
                    for cc in range(2):
                        nc.tensor.matmul(
                            o_ps[cc],
                            v_sb[mb][:, cc * 128:(cc + 1) * 128],
                            esl,
                            start=first, stop=last,
                        )
                # softmax denominator: DVE pairwise-sums the 4 key blocks,
                # then one ones-matmul per group accumulates Z in PSUM
                e01 = esp.tile([128, NT], BF16, tag="e01", name=f"e01_{nt}_{g}")
                e23 = esp.tile([128, NT], BF16, tag="e23", name=f"e23_{nt}_{g}")
                es = esp.tile([128, NT], BF16, tag="es", name=f"es_{nt}_{g}")
                nc.vector.tensor_add(e01, e_sb[:, 0:NT], e_sb[:, NT:2 * NT])
                nc.vector.tensor_add(e23, e_sb[:, 2 * NT:3 * NT], e_sb[:, 3 * NT:])
                nc.vector.tensor_add(es, e01, e23)
                nc.tensor.matmul(
                    z_ps, ones_col, es, start=(g == 0), stop=(g == NGRP - 1),
                )
                if g == NGRP - 1:
                    emit_tail(nt)

            # Pipelined one group deep: T-pack(i) ... O'/Z(i-1). exp(i) on ACT
            # hides under O'/Z(i-1) on PE; the per-tile tail (DVE-heavy) hides
            # under the next tile's first T-pack/O'Z groups.
            prev = None
            for nt in range(NTILES):
                nsl = slice(nt * NT, (nt + 1) * NT)
                for g in range(NGRP):
                    # 4-way row-packed score matmuls: strip j contracts its own
                    # 32 rows of the PE array concurrently (measured ~3x).
                    t_ps = pt.tile([128, GRP * NT], F32, tag="t", name=f"t{nt}_{g}")
                    for j in range(GRP):
                        nc.tensor.matmul(
                            t_ps[:, j * NT:(j + 1) * NT],
                            kt_stack[32 * j:32 * (j + 1), g, :],
                            qt_rep[32 * j:32 * (j + 1), nsl],
                            start=True, stop=True,
                            tile_position=(32 * j, 0),
                        )
                    e_sb = ep.tile([128, GRP * NT], BF16, tag="e", name=f"e{nt}_{g}")
                    nc.scalar.activation(out=e_sb, in_=t_ps, func=FT.Exp)
                    if prev is not None:
                        emit_oz(*prev)
                    prev = (nt, g, e_sb)
            emit_oz(*prev)

    _split_multiwaits(nc)
    return nc


def _split_multiwaits(nc: bass.Bass) -> None:
    """This container's walrus accepts at most ONE sync-wait per instruction
    (CoreV3GenImpl setupSyncWait). Tile emits multi-wait instructions; split
    the excess waits onto EventSemaphore carriers inserted just before the
    instruction on the same engine — same-engine program order makes this
    semantics-preserving."""
    import json as _json

    data = _json.loads(mybir.module_to_json_bytes(nc.m))
    uid = 0
    for fn in data["functions"]:
        for bb in fn["blocks"]:
            new = []
            for inst in bb["instructions"]:
                si = inst.get("sync_info")
                waits = (si or {}).get("on_wait") or []
                if len(waits) > 1:
                    for wcmd in waits[:-1]:
                        uid += 1
                        new.append({
                            "debug": inst.get("debug", 0),
                            "engine": inst["engine"],
                            "ins": [], "outs": [],
                            "name": f"syncw-{uid}",
                            "opcode": "EventSemaphore",
                            "sync_info": {"on_update": [], "on_wait": [wcmd]},
                        })
                    si["on_wait"] = [waits[-1]]
                new.append(inst)
            bb["instructions"] = new
    nc.m = mybir.module_from_json_bytes(_json.dumps(data).encode())


_NC = None


def _get_nc():
    global _NC
    if _NC is None:
        _NC = build_nc()
    return _NC


def _prep_maps(x, Wf, bf, Wg, bg, Wh, bh, Wo, bo):
    bft = ml_dtypes.bfloat16
    # V bias folds through the (linear) attention average into the output
    # projection: out = (O'/z) @ Wo + (bh @ Wo + bo)
    bo_prime = (Wo.T.astype(np.float64) @ bh.astype(np.float64)
                + bo.astype(np.float64)).astype(np.float32)
    shared = {
        "wf": np.ascontiguousarray(Wf.reshape(2, 128, KEY).astype(bft)),
        "wg": np.ascontiguousarray(Wg.reshape(2, 128, KEY).astype(bft)),
        "wh": np.ascontiguousarray(Wh.reshape(2, 128, C).astype(bft)),
        "wo": np.ascontiguousarray(Wo.reshape(2, 128, C).astype(bft)),
        "bfp": np.ascontiguousarray(bf.reshape(KEY, 1).astype(np.float32)),
        "bgp": np.ascontiguousarray(bg.reshape(KEY, 1).astype(np.float32)),
        "bop": np.ascontiguousarray(bo_prime.reshape(2, 128, 1)),
    }
    in_maps = []
    for b in range(B):
        xTb = np.ascontiguousarray(
            x[b].reshape(N, C).T.astype(bft).reshape(2, 128, N)
        )
        m = dict(shared)
        m["xT"] = xTb
        in_maps.append(m)
    return in_maps


def run(x, Wf, bf, Wg, bg, Wh, bh, Wo, bo, trace=False, **kw):
    x = np.asarray(x, dtype=np.float32)
    in_maps = _prep_maps(
        x, *(np.asarray(a, dtype=np.float32) for a in (Wf, bf, Wg, bg, Wh, bh, Wo, bo))
    )
    res = run_bass_kernel_spmd(_get_nc(), in_maps, list(range(B)), trace=trace, **kw)
    out = np.empty((B, H, W, C), dtype=np.float32)
    for b in range(B):
        oT = np.asarray(res.results[b]["outT"], dtype=np.float32).reshape(C, N)
        out[b] = oT.T.reshape(H, W, C)
    return out, res


def kernel(x, Wf, bf, Wg, bg, Wh, bh, Wo, bo):
    out, _ = run(x, Wf, bf, Wg, bg, Wh, bh, Wo, bo)
    return out

